# revision 6
# baseline (speedup 1.0000x reference)
"""Trainium2 Bass kernel for nn_ATTModule_44865228374086 (moe_routing).

Reference computation (per batch element b):
    pooled = mean(features[b], over H*W)                       # [C]
    h      = relu(pooled @ fc1_w[a] + fc1_b[a])                # [A, CH]
    expert = h @ fc2_w[a] + fc2_b[a]                           # [A, C]
    gate   = softmax(fc_w @ pooled + fc_b)                     # [A]
    mixed  = sum_a gate[a] * expert[a]                         # [C]
    out[b] = features[b] * (1 + sigmoid(mixed))                # [C, H, W]

Strategy: data-parallel over batch across 8 cores (128 samples each).
Per core, features are streamed HBM->SBUF once in groups of G=8 samples,
pooled on DVE (free-axis reduce), the tiny MLP runs on PE in bf16
(weights are host-permuted / pre-scaled by 1/196, w2 also by 0.5, and
kept SBUF-resident), the per-channel scale 1+sigmoid(mixed) =
1.5 + 0.5*tanh(mixed/2) is computed on ACT, and the scale is applied in
place (ACT activation-copy / GpSimd tensor_scalar) before streaming out.

Engine layout (HW-validated): DVE runs pools, the tiny softmax
reciprocal, and 6 of 8 apply samples; ACT runs exp/relu/tanh/
psum-copies/scl and the other 2 applies; all feature loads AND stores
dispatch from the SP (sync) ring.  Two HW findings the cost model
misses: GpSimd elementwise ops and ACT-ring-issued stores are both
pathologically slow on real TRN2 (~3x whole-kernel regressions), so
neither is used.  fc1/fc2 weights are fp8e4 (halves their SBUF to
32KB/partition), which buys FBUFS=13 feature tiles -- enough lookahead
to keep the DMA engines ~95% busy.  The softmax is never materialized:
relu is scaled by unnormalized exp(logits) (fused sum via the Exp op's
accum_out) and the denominator folds into the tanh scale as a
per-sample reciprocal; the 1/HW pooling mean is applied at the P16
downcast so the fp8 weights stay at their natural scale.

SBUF layout: partition p holds channels 8p..8p+7, so each per-sample DMA
is 128 partitions x 6272 contiguous bytes.

Biases are all zero for this problem (spec fill: zeros); the default
program omits them. If nonzero biases are ever passed, a second program
variant that adds them via K=1 matmuls is built on the fly.
"""

import numpy as np
import ml_dtypes

import concourse.bacc as bacc
import concourse.tile as tile
from concourse import mybir
from concourse.bass_utils import axon_active, run_bass_kernel_spmd
from concourse.masks import make_identity

B, C, H, W = 1024, 1024, 14, 14
HW = H * W            # 196
A = 8                 # experts
CH = C // 4           # 256
ACH = A * CH          # 2048
NCORES = 8
import os

BS = B // NCORES      # 128 samples per core
K8 = C // 128         # 8 channels per partition
G = 8                 # samples per MLP group
TPB = int(os.environ.get("KTPB", "2"))   # samples per feature tile
NT = BS // TPB        # tiles
TPG = G // TPB        # tiles per group
NG = BS // G          # 16 groups
FBUFS = int(os.environ.get("KFBUFS", "13"))  # feature-tile pool depth
SCHEME = os.environ.get("KSCHEME", "base")   # apply/store engine scheme
NDVE = int(os.environ.get("KNDVE", "6"))     # samples applied on DVE (base)
WF8 = os.environ.get("KWF8", "1") == "1"     # fp8 fc1/fc2 weights (SBUF headroom)
DVE_APPLY_TILES = ()  # which tiles of a group apply on DVE (rest ACT)
TPB_COPY = 4          # hT transposes batched per PSUM->SBUF copy
COLTILE = False       # column-tile fc2 across PE strips (HW-neutral)
PB_STORES = os.environ.get("KPBS", "0") == "1"  # store per sample vs per tile
FSPLIT = os.environ.get("KFSPLIT", "1") == "1"  # split sample-0 apply DVE+ACT
POOL_APPLY_TILES = (2, 3)  # which tiles of a group apply on GpSimd

F32 = mybir.dt.float32
BF16 = mybir.dt.bfloat16
FP8 = mybir.dt.float8e4
WDT = FP8 if WF8 else BF16  # fc1/fc2 weight dtype
# Feature I/O dtype.  The kernel is HBM-bound (~358 GB/s/NC limit; f32
# streams 205 MB/core), and the 2e-2 rel-err budget dwarfs bf16's ~4e-3
# rounding, so features stream HBM<->SBUF as bf16 (host casts both ways),
# halving HBM traffic.
FDT = {"bf16": BF16, "f32": F32}[os.environ.get("KFDT", "bf16")]
if FDT is BF16 and "KFBUFS" not in os.environ:
    FBUFS = 24  # bf16 feature tiles are half size; deepen the pool
AF = mybir.ActivationFunctionType
ALU = mybir.AluOpType
AX = mybir.AxisListType

_NC_CACHE = {}


def _emit(tc, nc, feat, w1, wg, w2, b1h, b1g, b2, out, with_bias, repeat=1):
    # DRAM views: partition p <- channels 8p..8p+7
    fv = feat.rearrange("b (p k) hw -> p b k hw", k=K8)   # [128, BS, 8, 196]
    ov = out.rearrange("b (p k) hw -> p b k hw", k=K8)

    with (
        tc.tile_pool(name="pw", bufs=1) as pw,
        tc.tile_pool(name="pf", bufs=FBUFS) as pf,
        tc.tile_pool(name="pp", bufs=2) as pp,
        tc.tile_pool(name="ph1", bufs=1) as ph1,
        tc.tile_pool(name="ph2", bufs=2) as ph2,
        tc.tile_pool(name="psh", bufs=1, space="PSUM") as psh,
        tc.tile_pool(name="psm", bufs=1, space="PSUM") as psm,
        tc.tile_pool(name="pss", bufs=2, space="PSUM") as pss,
    ):
        # Resident weights / constants.
        w1_sb = pw.tile([128, K8, ACH], WDT)
        nc.scalar.dma_start(out=w1_sb, in_=w1)
        wg_sb = pw.tile([128, K8, A], BF16)
        nc.scalar.dma_start(out=wg_sb, in_=wg)
        w2_sb = pw.tile([128, ACH // 128, C], WDT)
        nc.scalar.dma_start(out=w2_sb, in_=w2)
        id16 = pw.tile([128, 128], BF16)
        make_identity(nc, id16)
        if with_bias:
            id32 = pw.tile([128, 128], F32)
            make_identity(nc, id32)
            b1h_sb = pw.tile([1, ACH], BF16)
            nc.sync.dma_start(out=b1h_sb, in_=b1h)
            b1g_sb = pw.tile([1, A], BF16)
            nc.sync.dma_start(out=b1g_sb, in_=b1g)
            b2_sb = pw.tile([A, C], BF16)
            nc.sync.dma_start(out=b2_sb, in_=b2)
            ones16 = pw.tile([1, G], BF16)
            nc.vector.memset(ones16, 1.0)

        import contextlib
        loop_cm = tc.For_i(0, repeat, 1) if repeat > 1 else contextlib.nullcontext()
        with loop_cm:
            _emit_groups(
                tc, nc, fv, ov, pf, pp, ph1, ph2, psh, psm, pss,
                w1_sb, wg_sb, w2_sb, id16,
                (id32, b1h_sb, b1g_sb, b2_sb, ones16) if with_bias else None,
                with_bias,
            )


def _emit_groups(tc, nc, fv, ov, pf, pp, ph1, ph2, psh, psm, pss,
                 w1_sb, wg_sb, w2_sb, id16, bias_tiles, with_bias):
        if with_bias:
            id32, b1h_sb, b1g_sb, b2_sb, ones16 = bias_tiles
        for g in range(NG):
            # ---- load + pool ----
            Pg = pp.tile([128, K8 * G], F32, tag="Pg")
            Pview = Pg.rearrange("p (k g) -> p g k", g=G)  # [128, G, 8]
            ftiles = []
            for t in range(TPG):
                b0 = g * G + t * TPB
                ft = pf.tile([128, TPB, K8, HW], FDT, tag="ft")
                nc.sync.dma_start(out=ft, in_=fv[:, b0 : b0 + TPB])
                # Per-sample reduces: smaller DVE quanta schedule around the
                # latency-critical MLP ops instead of blocking them.
                for bb in range(TPB):
                    nc.vector.tensor_reduce(
                        out=Pview[:, t * TPB + bb : t * TPB + bb + 1, :],
                        in_=ft[:, bb],
                        axis=AX.X,
                        op=ALU.add,
                    )
                ftiles.append(ft)
            # The 1/HW pooling mean is applied here (not folded into the
            # weights: fp8 weights would underflow at the 1/196 scale).
            P16 = pp.tile([128, K8 * G], BF16, tag="P16")
            nc.vector.tensor_scalar_mul(out=P16, in0=Pg, scalar1=1.0 / HW)

            # ---- gating logits first (softmax overlaps the fc1 matmuls) ----
            gps = pss.tile([G, A], F32, tag="pst")
            for k in range(K8):
                nc.tensor.matmul(
                    out=gps,
                    lhsT=P16[:, k * G : (k + 1) * G],
                    rhs=wg_sb[:, k, :],
                    start=(k == 0),
                    stop=(k == K8 - 1) and not with_bias,
                )
            if with_bias:
                nc.tensor.matmul(
                    out=gps, lhsT=ones16, rhs=b1g_sb, start=False, stop=True
                )

            # ---- fc1: psum_h[b, a*CH+ch] ----
            hps = psh.tile([G, ACH], F32)
            for n in range(ACH // 512):
                for k in range(K8):
                    nc.tensor.matmul(
                        out=hps[:, n * 512 : (n + 1) * 512],
                        lhsT=P16[:, k * G : (k + 1) * G],
                        rhs=w1_sb[:, k, n * 512 : (n + 1) * 512],
                        start=(k == 0),
                        stop=(k == K8 - 1) and not with_bias,
                    )
                if with_bias:
                    nc.tensor.matmul(
                        out=hps[:, n * 512 : (n + 1) * 512],
                        lhsT=ones16,
                        rhs=b1h_sb[:, n * 512 : (n + 1) * 512],
                        start=False,
                        stop=True,
                    )

            # ---- unnormalized gate weights ge = exp(logits); the softmax
            # denominator gs is folded into the final tanh scale (w2 carries
            # the 0.5 of tanh(mixed/2), so scale = 1/gs).  exp+sum fuse into
            # one ACT op; only the tiny reciprocal runs on DVE, early in the
            # chain, so next group's pools aren't blocked behind it.
            ge = ph2.tile([G, A], F32, tag="ge")
            gs = ph2.tile([G, 1], F32, tag="gs")
            nc.scalar.activation(out=ge, in_=gps, func=AF.Exp, accum_out=gs)
            gi = ph2.tile([G, 1], F32, tag="gi")
            nc.vector.reciprocal(out=gi, in_=gs)

            if with_bias:
                gtp = pss.tile([A, G], F32, tag="pst")
                nc.tensor.transpose(gtp, ge, id32[0:G, 0:G])
                gt16 = ph2.tile([A, G], BF16, tag="gt16")
                nc.vector.tensor_copy(out=gt16, in_=gtp)

            # ---- h' = ge*relu(h) -> transpose -> fc2, pipelined per chunk.
            # PE executes in emission order, so interleave the transposes with
            # the fc2 matmuls; relu and psum->sbuf copies run on ACT so DVE
            # stays free for next group's pools.
            h16 = ph1.tile([G, ACH], BF16, tag="h16")
            hT = pp.tile([128, (ACH // 128) * G], BF16, tag="hT")
            mps = psm.tile([G, C], F32, tag="mps")
            nt = ACH // 128
            for t0 in range(0, nt, TPB_COPY):
                for t in range(t0, t0 + TPB_COPY):
                    if t % 2 == 0:
                        a = t // 2
                        nc.scalar.activation(
                            out=h16[:, a * CH : (a + 1) * CH],
                            in_=hps[:, a * CH : (a + 1) * CH],
                            func=AF.Relu,
                            scale=ge[:, a : a + 1],
                        )
                tp = pss.tile([128, TPB_COPY * G], BF16, tag="pst")
                for j in range(TPB_COPY):
                    nc.tensor.transpose(
                        tp[:, j * G : (j + 1) * G],
                        h16[:, (t0 + j) * 128 : (t0 + j + 1) * 128],
                        id16[0:G, 0:G],
                    )
                nc.scalar.activation(
                    out=hT[:, t0 * G : (t0 + TPB_COPY) * G], in_=tp, func=AF.Copy
                )
                for t in range(t0, t0 + TPB_COPY):
                    for n in range(C // 512):
                        nc.tensor.matmul(
                            out=mps[:, n * 512 : (n + 1) * 512],
                            lhsT=hT[:, t * G : (t + 1) * G],
                            rhs=w2_sb[:, t, n * 512 : (n + 1) * 512],
                            start=(t == 0),
                            stop=(t == nt - 1) and not with_bias,
                        )
            if with_bias:
                for n in range(C // 512):
                    nc.tensor.matmul(
                        out=mps[:, n * 512 : (n + 1) * 512],
                        lhsT=gt16,
                        rhs=b2_sb[:, n * 512 : (n + 1) * 512],
                        start=False,
                        stop=True,
                    )

            # ---- scale = 1 + sigmoid(mixed) = 1.5 + 0.5*tanh(mixed/2).
            # w2 is host-scaled by 0.5, so mps = mixed_unnorm/2 and the
            # per-sample softmax denominator folds into the tanh scale.
            scl = pp.tile([128, K8 * G], F32, tag="scl")
            sp = pss.tile([128, K8 * G], BF16, tag="pst")
            mx = ph1.tile([G, C], BF16, tag="mx")
            nc.scalar.activation(out=mx, in_=mps, func=AF.Tanh, scale=gi[:, 0:1])
            mxv = mx.rearrange("g (p k) -> g k p", k=K8)  # [G, 8, 128]
            for k in range(K8):
                nc.tensor.transpose(
                    sp[:, k * G : (k + 1) * G], mxv[:, k, :], id16[0:G, 0:G]
                )
            nc.scalar.activation(
                out=scl, in_=sp, func=AF.Copy, scale=0.5, bias=1.5
            )

            # ---- apply scale in place, store.
            # ACT applies samples 0..3 (tiles 0,1); GpSimd applies samples
            # 4..7 (tiles 2,3), concurrently.  Each engine dispatches the
            # stores of the OTHER engine's samples, one sample behind, so a
            # store's apply has always completed by the time the dispatching
            # sequencer reaches it (an unmet DMA wait holds the sequencer).
            # Loads live alone on the SP ring; ACT stores use the second
            # HWDGE ring; GpSimd stores go out via SWDGE.
            def _apply(eng, bcol):
                t, bb = divmod(bcol, TPB)
                for k in range(K8):
                    sl = ftiles[t][:, bb, k, :]
                    s1 = scl[:, k * G + bcol : k * G + bcol + 1]
                    if eng == "act":
                        nc.scalar.activation(out=sl, in_=sl, func=AF.Copy, scale=s1)
                    elif eng == "dve":
                        nc.vector.tensor_scalar_mul(out=sl, in0=sl, scalar1=s1)
                    else:
                        nc.gpsimd.tensor_scalar_mul(out=sl, in0=sl, scalar1=s1)

            def _store(eng, bcol):
                t, bb = divmod(bcol, TPB)
                dma = {
                    "act": nc.scalar.dma_start,
                    "sync": nc.sync.dma_start,
                    "pool": nc.gpsimd.dma_start,
                }[eng]
                dma(out=ov[:, g * G + bcol], in_=ftiles[t][:, bb])

            HALF = G // 2
            if SCHEME == "v6":
                # v1 but with a fast wave turn-on: sample 0's apply is split
                # across ACT and GpSimd (parallel halves, ~1.5us), and its
                # store dispatches from GpSimd's SWDGE immediately, ~3us
                # after scl instead of ~5.5.  Remaining samples follow v1.
                t0, bb0 = divmod(0, TPB)
                for k in range(K8):
                    sl = ftiles[t0][:, bb0, k, :]
                    s1 = scl[:, k * G : k * G + 1]
                    if k < K8 // 2:
                        nc.scalar.activation(out=sl, in_=sl, func=AF.Copy, scale=s1)
                    else:
                        nc.gpsimd.tensor_scalar_mul(out=sl, in0=sl, scalar1=s1)
                _store("pool", 0)
                for b in range(1, G):
                    _apply("act" if b < HALF else "pool", b)
                    _store("act", b)
            elif SCHEME == "v7":
                # v6's split first-apply + ACT/GpSimd apply halves, but all
                # stores dispatch from the SP ring like the original kernel
                # (ACT-issued stores measured pathologically slow on HW).
                t0, bb0 = divmod(0, TPB)
                for k in range(K8):
                    sl = ftiles[t0][:, bb0, k, :]
                    s1 = scl[:, k * G : k * G + 1]
                    if k < K8 // 2:
                        nc.scalar.activation(out=sl, in_=sl, func=AF.Copy, scale=s1)
                    else:
                        nc.gpsimd.tensor_scalar_mul(out=sl, in0=sl, scalar1=s1)
                _store("sync", 0)
                for b in range(1, G):
                    _apply("act" if b < HALF else "pool", b)
                    _store("sync", b)
            elif SCHEME == "v2":
                # ACT applies 0..3, GpSimd applies 4..7; each stores the
                # other's samples one sample behind.
                for i in range(HALF):
                    _apply("act", i)
                    _apply("pool", HALF + i)
                    if i >= 1:
                        _store("act", HALF + i - 1)
                        _store("pool", i - 1)
                _store("act", 2 * HALF - 1)
                _store("pool", HALF - 1)
            elif SCHEME == "v1":
                # ACT applies+stores 0..3, GpSimd applies 4..7, ACT stores them.
                for b in range(G):
                    _apply("act" if b < HALF else "pool", b)
                    _store("act", b)
            elif SCHEME == "base":
                # DVE + ACT applies (GpSimd compute measured pathologically
                # slow on HW), stores on the sync ring.  NDVE picks how many
                # samples apply on DVE; ACT (which also runs the MLP glue)
                # takes the rest.  With FSPLIT, sample 0's eight slices are
                # split across DVE and ACT so the wave's first store
                # dispatches ~1.4us after scl instead of ~2.8us.
                start_b = 0
                if FSPLIT:
                    for k in range(K8):
                        sl = ftiles[0][:, 0, k, :]
                        s1 = scl[:, k * G : k * G + 1]
                        if k < K8 // 2:
                            nc.vector.tensor_scalar_mul(out=sl, in0=sl, scalar1=s1)
                        else:
                            nc.scalar.activation(out=sl, in_=sl, func=AF.Copy, scale=s1)
                    if PB_STORES:
                        _store("sync", 0)
                    start_b = 1
                for b in range(start_b, G):
                    _apply("dve" if b < NDVE else "act", b)
                    if PB_STORES:
                        _store("sync", b)
                if not PB_STORES:
                    for t in range(TPG):
                        b0 = g * G + t * TPB
                        nc.sync.dma_start(out=ov[:, b0 : b0 + TPB], in_=ftiles[t])
            elif SCHEME == "v4":
                # ACT applies s0..3, GpSimd applies s4..7.  GpSimd's SWDGE
                # dispatches the stores of ACT's samples interleaved with its
                # own applies (first store fires ~3us after scl); ACT
                # dispatches the stores of GpSimd's samples at its tail.
                _store("pool", 0)
                for i in range(HALF):
                    _apply("act", i)
                    _apply("pool", HALF + i)
                    if i >= 1:
                        _store("pool", i)
                for i in range(HALF):
                    _store("act", HALF + i)
            elif SCHEME == "v3":
                # like v2 but stores dispatch from the engine that applied,
                # one sample behind its own stream (exec-queue drain hidden).
                for i in range(HALF):
                    _apply("act", i)
                    _apply("pool", HALF + i)
                    if i >= 1:
                        _store("act", i - 1)
                        _store("pool", HALF + i - 1)
                _store("act", HALF - 1)
                _store("pool", 2 * HALF - 1)
            else:
                raise ValueError(SCHEME)


def build_nc(with_bias=False, repeat=1):
    nc = bacc.Bacc(
        "TRN2",
        target_bir_lowering=False,
        debug=not axon_active(),
        num_devices=NCORES,
    )
    feat = nc.dram_tensor("features", [BS, C, HW], FDT, kind="ExternalInput").ap()
    w1 = nc.dram_tensor("w1", [128, K8, ACH], WDT, kind="ExternalInput").ap()
    wg = nc.dram_tensor("wg", [128, K8, A], BF16, kind="ExternalInput").ap()
    w2 = nc.dram_tensor("w2", [128, ACH // 128, C], WDT, kind="ExternalInput").ap()
    b1h = b1g = b2 = None
    if with_bias:
        b1h = nc.dram_tensor("b1h", [1, ACH], BF16, kind="ExternalInput").ap()
        b1g = nc.dram_tensor("b1g", [1, A], BF16, kind="ExternalInput").ap()
        b2 = nc.dram_tensor("b2", [A, C], BF16, kind="ExternalInput").ap()
    out = nc.dram_tensor("out", [BS, C, HW], FDT, kind="ExternalOutput").ap()

    with tile.TileContext(nc) as tc:
        _emit(tc, nc, feat, w1, wg, w2, b1h, b1g, b2, out, with_bias, repeat)
    nc.compile()
    return nc


def _get_nc(with_bias=False):
    key = "bias" if with_bias else "nobias"
    if key not in _NC_CACHE:
        _NC_CACHE[key] = build_nc(with_bias)
    return _NC_CACHE[key]


def prep_host_inputs(features, fc_w, fc_b, fc1_w, fc1_b, fc2_w, fc2_b):
    """Returns (per-core features list, shared weight dict, with_bias)."""
    bf16 = ml_dtypes.bfloat16
    wdt = mybir.dt.np(WDT)

    fc1_w = np.asarray(fc1_w, dtype=np.float32)
    fc_w = np.asarray(fc_w, dtype=np.float32)
    fc2_w = np.asarray(fc2_w, dtype=np.float32)
    fc_b = np.asarray(fc_b, dtype=np.float32)
    fc1_b = np.asarray(fc1_b, dtype=np.float32)
    fc2_b = np.asarray(fc2_b, dtype=np.float32)
    with_bias = bool(np.any(fc_b) or np.any(fc1_b) or np.any(fc2_b))

    # The 1/HW spatial-mean factor is applied on-chip at the P16 copy, so
    # the weights stay at their natural scale (fp8-safe).
    # w1[p, k, a*CH+ch] = fc1_w[a, 8p+k, ch]
    w1 = (
        np.transpose(fc1_w, (1, 0, 2)).reshape(C, ACH).reshape(128, K8, ACH)
    ).astype(wdt)
    # wg[p, k, a] = fc_w[a, 8p+k]
    wg = (fc_w.T.reshape(128, K8, A)).astype(bf16)
    # w2[p, t, c] = fc2_w.reshape(ACH, C)[t*128+p, c] * 0.5 -- the 0.5 of
    # tanh(mixed/2) is folded into w2 so the softmax denominator 1/gs is the
    # only runtime factor in the tanh scale.
    w2 = np.ascontiguousarray(
        fc2_w.reshape(ACH, C).reshape(ACH // 128, 128, C).transpose(1, 0, 2) * 0.5
    ).astype(wdt)
    weights = {
        "w1": np.ascontiguousarray(w1),
        "wg": np.ascontiguousarray(wg),
        "w2": w2,
    }
    if with_bias:
        weights["b1h"] = fc1_b.reshape(1, ACH).astype(bf16)
        weights["b1g"] = fc_b.reshape(1, A).astype(bf16)
        weights["b2"] = (fc2_b * 0.5).astype(bf16)
    f = np.ascontiguousarray(np.asarray(features, dtype=np.float32)).reshape(B, C, HW)
    if FDT is not F32:
        f = f.astype(mybir.dt.np(FDT))
    shards = [f[i * BS : (i + 1) * BS] for i in range(NCORES)]
    return shards, weights, with_bias


def run(inputs, trace=False, trace_kwargs=None):
    shards, weights, with_bias = prep_host_inputs(**inputs)
    nc = _get_nc(with_bias)
    in_maps = [dict(weights, features=shards[i]) for i in range(NCORES)]
    res = run_bass_kernel_spmd(
        nc,
        in_maps,
        core_ids=list(range(NCORES)),
        trace=trace,
        **(trace_kwargs or {}),
    )
    out = np.concatenate([res.results[i]["out"] for i in range(NCORES)], axis=0)
    return out.reshape(B, C, H, W), res


def kernel(**inputs):
    out, _ = run(inputs, trace=False)
    return np.asarray(out, dtype=np.float32)



# revision 24
# speedup vs baseline: 1.5609x; 1.5609x over previous
"""Trainium2 Bass kernel for nn_ATTModule_44865228374086 (moe_routing).

Reference computation (per batch element b):
    pooled = mean(features[b], over H*W)                       # [C]
    h      = relu(pooled @ fc1_w[a] + fc1_b[a])                # [A, CH]
    expert = h @ fc2_w[a] + fc2_b[a]                           # [A, C]
    gate   = softmax(fc_w @ pooled + fc_b)                     # [A]
    mixed  = sum_a gate[a] * expert[a]                         # [C]
    out[b] = features[b] * (1 + sigmoid(mixed))                # [C, H, W]

Strategy: data-parallel over batch across 8 cores (128 samples each).
Per core, features are streamed HBM->SBUF once in groups of G=8 samples,
pooled on DVE (free-axis reduce), the tiny MLP runs on PE in bf16
(weights are host-permuted / pre-scaled by 1/196, w2 also by 0.5, and
kept SBUF-resident), the per-channel scale 1+sigmoid(mixed) =
1.5 + 0.5*tanh(mixed/2) is computed on ACT, and the scale is applied in
place (ACT activation-copy / GpSimd tensor_scalar) before streaming out.

2026-08-09 round 2 (HW-measured; see memory/trn2-hbm-roofline-facts.md):
the rel-err budget (2e-2 vs our ~7e-3) lets features stream as BF16
both ways (host casts), halving HBM bytes; features are host-repacked
partition-major ([128, BS, K8, HW]) so each (partition, tile) DMA is
one contiguous 12.5KB run -- descriptor count, not bytes, bound the
bf16 kernel (173ns fixed/descriptor).  The kernel is now DVE-bound (HW
DVE ~1.7x the cost model): pooling folds hw pairwise at the 2x bf16
tensor_tensor rate before the 1x-rate reduce (KPOOLADD levels).
f32 baseline 654us -> bf16 503 -> +repack 422us.

Engine layout (HW-validated): DVE runs pools, the tiny softmax
reciprocal, and 6 of 8 apply samples; ACT runs exp/relu/tanh/
psum-copies/scl and the other 2 applies; all feature loads AND stores
dispatch from the SP (sync) ring.  Two HW findings the cost model
misses: GpSimd elementwise ops and ACT-ring-issued stores are both
pathologically slow on real TRN2 (~3x whole-kernel regressions), so
neither is used.  fc1/fc2 weights are fp8e4 (halves their SBUF to
32KB/partition), which buys FBUFS=13 feature tiles -- enough lookahead
to keep the DMA engines ~95% busy.  The softmax is never materialized:
relu is scaled by unnormalized exp(logits) (fused sum via the Exp op's
accum_out) and the denominator folds into the tanh scale as a
per-sample reciprocal; the 1/HW pooling mean is applied at the P16
downcast so the fp8 weights stay at their natural scale.

SBUF layout: partition p holds channels 8p..8p+7, so each per-sample DMA
is 128 partitions x 6272 contiguous bytes.

Biases are all zero for this problem (spec fill: zeros); the default
program omits them. If nonzero biases are ever passed, a second program
variant that adds them via K=1 matmuls is built on the fly.
"""

import numpy as np
import ml_dtypes

import concourse.bacc as bacc
import concourse.tile as tile
from concourse import mybir
from concourse.bass_utils import axon_active, run_bass_kernel_spmd
from concourse.masks import make_identity

B, C, H, W = 1024, 1024, 14, 14
HW = H * W            # 196
A = 8                 # experts
CH = C // 4           # 256
ACH = A * CH          # 2048
NCORES = 8
import os

BS = B // NCORES      # 128 samples per core
K8 = C // 128         # 8 channels per partition
G = 8                 # samples per MLP group
TPB = int(os.environ.get("KTPB", "2"))   # samples per feature tile
NT = BS // TPB        # tiles
TPG = G // TPB        # tiles per group
NG = BS // G          # 16 groups
FBUFS = int(os.environ.get("KFBUFS", "13"))  # feature-tile pool depth
SCHEME = os.environ.get("KSCHEME", "base")   # apply/store engine scheme
NDVE = int(os.environ.get("KNDVE", "6"))     # samples applied on DVE (base)
WF8 = os.environ.get("KWF8", "1") == "1"     # fp8 fc1/fc2 weights (SBUF headroom)
DVE_APPLY_TILES = ()  # which tiles of a group apply on DVE (rest ACT)
TPB_COPY = 4          # hT transposes batched per PSUM->SBUF copy
COLTILE = False       # column-tile fc2 across PE strips (HW-neutral)
PB_STORES = os.environ.get("KPBS", "0") == "1"  # store per sample vs per tile
FSPLIT = os.environ.get("KFSPLIT", "1") == "1"  # split sample-0 apply DVE+ACT
POOL_APPLY_TILES = (2, 3)  # which tiles of a group apply on GpSimd

F32 = mybir.dt.float32
BF16 = mybir.dt.bfloat16
FP8 = mybir.dt.float8e4
WDT = FP8 if WF8 else BF16  # fc1/fc2 weight dtype
# Feature I/O dtype.  The kernel is HBM-bound (~358 GB/s/NC limit; f32
# streams 205 MB/core), and the 2e-2 rel-err budget dwarfs bf16's ~4e-3
# rounding, so features stream HBM<->SBUF as bf16 (host casts both ways),
# halving HBM traffic.
FDT = {"bf16": BF16, "f32": F32}[os.environ.get("KFDT", "bf16")]
# Host-repacked partition-major feature layout [128, BS, K8*HW]: each
# (partition, tile) is ONE contiguous DRAM run, so a TPB-sample DMA is
# 128 descriptors of TPB*3136B instead of 128*TPB of 3136B.  HW showed
# ~173ns fixed cost per descriptor (f32 6272B descs: 319ns each; bf16
# 3136B: 246ns) -- descriptor count, not bytes, bound the bf16 kernel.
REPACK = os.environ.get("KREPACK", "1") == "1"
# Pairwise-add pooling: bf16 tensor_tensor adds (2x_1p DVE rate) fold
# hw 196->98(->49) before the full-rate reduce; each level halves the
# elements the 1x-rate reduce sees.  0=off, 1=one level, 2=two levels.
POOLADD = int(os.environ.get("KPOOLADD", "1")) if FDT is BF16 else 0
# Which ring dispatches the feature stores (PB_STORES=0 path): HWDGE
# DMAs execute FIFO per ring, so stores sharing the SP ring with loads
# head-of-line block them; "pool" (SWDGE) / "act" (second HWDGE ring)
# give stores their own FIFO into the same 16 SDMA engines.
SRING = os.environ.get("KSRING", "sync")
if REPACK and "KTPB" not in os.environ:
    TPB = 4  # bigger contiguous runs per descriptor
    NT = BS // TPB
    TPG = G // TPB
if "KFBUFS" not in os.environ:
    # ~150KB/partition of feature-tile lookahead regardless of dtype/TPB
    FBUFS = max(3, (150 * 1024) // (TPB * K8 * HW * (2 if FDT is BF16 else 4)))
AF = mybir.ActivationFunctionType
ALU = mybir.AluOpType
AX = mybir.AxisListType

_NC_CACHE = {}


def _emit(tc, nc, feat, w1, wg, w2, b1h, b1g, b2, out, with_bias, repeat=1):
    # DRAM views: partition p <- channels 8p..8p+7
    if REPACK:
        fv, ov = feat, out  # already [128, BS, K8, HW] partition-major
    else:
        fv = feat.rearrange("b (p k) hw -> p b k hw", k=K8)  # [128, BS, 8, 196]
        ov = out.rearrange("b (p k) hw -> p b k hw", k=K8)

    with (
        tc.tile_pool(name="pw", bufs=1) as pw,
        tc.tile_pool(name="pf", bufs=FBUFS) as pf,
        tc.tile_pool(name="pha", bufs=2) as pha,
        tc.tile_pool(name="pp", bufs=2) as pp,
        tc.tile_pool(name="ph1", bufs=1) as ph1,
        tc.tile_pool(name="ph2", bufs=2) as ph2,
        tc.tile_pool(name="psh", bufs=1, space="PSUM") as psh,
        tc.tile_pool(name="psm", bufs=1, space="PSUM") as psm,
        tc.tile_pool(name="pss", bufs=2, space="PSUM") as pss,
    ):
        # Resident weights / constants.
        w1_sb = pw.tile([128, K8, ACH], WDT)
        nc.scalar.dma_start(out=w1_sb, in_=w1)
        wg_sb = pw.tile([128, K8, A], BF16)
        nc.scalar.dma_start(out=wg_sb, in_=wg)
        w2_sb = pw.tile([128, ACH // 128, C], WDT)
        nc.scalar.dma_start(out=w2_sb, in_=w2)
        id16 = pw.tile([128, 128], BF16)
        make_identity(nc, id16)
        if with_bias:
            id32 = pw.tile([128, 128], F32)
            make_identity(nc, id32)
            b1h_sb = pw.tile([1, ACH], BF16)
            nc.sync.dma_start(out=b1h_sb, in_=b1h)
            b1g_sb = pw.tile([1, A], BF16)
            nc.sync.dma_start(out=b1g_sb, in_=b1g)
            b2_sb = pw.tile([A, C], BF16)
            nc.sync.dma_start(out=b2_sb, in_=b2)
            ones16 = pw.tile([1, G], BF16)
            nc.vector.memset(ones16, 1.0)

        import contextlib
        loop_cm = tc.For_i(0, repeat, 1) if repeat > 1 else contextlib.nullcontext()
        with loop_cm:
            _emit_groups(
                tc, nc, fv, ov, pf, pha, pp, ph1, ph2, psh, psm, pss,
                w1_sb, wg_sb, w2_sb, id16,
                (id32, b1h_sb, b1g_sb, b2_sb, ones16) if with_bias else None,
                with_bias,
            )


def _emit_groups(tc, nc, fv, ov, pf, pha, pp, ph1, ph2, psh, psm, pss,
                 w1_sb, wg_sb, w2_sb, id16, bias_tiles, with_bias):
        if with_bias:
            id32, b1h_sb, b1g_sb, b2_sb, ones16 = bias_tiles
        for g in range(NG):
            # ---- load + pool ----
            Pg = pp.tile([128, K8 * G], F32, tag="Pg")
            Pview = Pg.rearrange("p (k g) -> p g k", g=G)  # [128, G, 8]
            ftiles = []
            for t in range(TPG):
                b0 = g * G + t * TPB
                ft = pf.tile([128, TPB, K8, HW], FDT, tag="ft")
                nc.sync.dma_start(out=ft, in_=fv[:, b0 : b0 + TPB])
                # Per-sample reduces: smaller DVE quanta schedule around the
                # latency-critical MLP ops instead of blocking them.
                for bb in range(TPB):
                    red_in = ft[:, bb]
                    if POOLADD >= 1:
                        # hw 196 -> 98 at the 2x bf16 DVE rate before the
                        # full-rate reduce: ~1.35us/sample vs 1.69us direct.
                        hf = pha.tile([128, K8, HW // 2], FDT, tag="hf")
                        nc.vector.tensor_tensor(
                            out=hf,
                            in0=ft[:, bb, :, 0 : HW // 2],
                            in1=ft[:, bb, :, HW // 2 : HW],
                            op=ALU.add,
                        )
                        red_in = hf
                    if POOLADD >= 2:
                        hq = pha.tile([128, K8, HW // 4], FDT, tag="hq")
                        nc.vector.tensor_tensor(
                            out=hq,
                            in0=hf[:, :, 0 : HW // 4],
                            in1=hf[:, :, HW // 4 : HW // 2],
                            op=ALU.add,
                        )
                        red_in = hq
                    nc.vector.tensor_reduce(
                        out=Pview[:, t * TPB + bb : t * TPB + bb + 1, :],
                        in_=red_in,
                        axis=AX.X,
                        op=ALU.add,
                    )
                ftiles.append(ft)
            # The 1/HW pooling mean is applied here (not folded into the
            # weights: fp8 weights would underflow at the 1/196 scale).
            P16 = pp.tile([128, K8 * G], BF16, tag="P16")
            nc.vector.tensor_scalar_mul(out=P16, in0=Pg, scalar1=1.0 / HW)

            # ---- gating logits first (softmax overlaps the fc1 matmuls) ----
            gps = pss.tile([G, A], F32, tag="pst")
            for k in range(K8):
                nc.tensor.matmul(
                    out=gps,
                    lhsT=P16[:, k * G : (k + 1) * G],
                    rhs=wg_sb[:, k, :],
                    start=(k == 0),
                    stop=(k == K8 - 1) and not with_bias,
                )
            if with_bias:
                nc.tensor.matmul(
                    out=gps, lhsT=ones16, rhs=b1g_sb, start=False, stop=True
                )

            # ---- fc1: psum_h[b, a*CH+ch] ----
            hps = psh.tile([G, ACH], F32)
            for n in range(ACH // 512):
                for k in range(K8):
                    nc.tensor.matmul(
                        out=hps[:, n * 512 : (n + 1) * 512],
                        lhsT=P16[:, k * G : (k + 1) * G],
                        rhs=w1_sb[:, k, n * 512 : (n + 1) * 512],
                        start=(k == 0),
                        stop=(k == K8 - 1) and not with_bias,
                    )
                if with_bias:
                    nc.tensor.matmul(
                        out=hps[:, n * 512 : (n + 1) * 512],
                        lhsT=ones16,
                        rhs=b1h_sb[:, n * 512 : (n + 1) * 512],
                        start=False,
                        stop=True,
                    )

            # ---- unnormalized gate weights ge = exp(logits); the softmax
            # denominator gs is folded into the final tanh scale (w2 carries
            # the 0.5 of tanh(mixed/2), so scale = 1/gs).  exp+sum fuse into
            # one ACT op; only the tiny reciprocal runs on DVE, early in the
            # chain, so next group's pools aren't blocked behind it.
            ge = ph2.tile([G, A], F32, tag="ge")
            gs = ph2.tile([G, 1], F32, tag="gs")
            nc.scalar.activation(out=ge, in_=gps, func=AF.Exp, accum_out=gs)
            gi = ph2.tile([G, 1], F32, tag="gi")
            nc.vector.reciprocal(out=gi, in_=gs)

            if with_bias:
                gtp = pss.tile([A, G], F32, tag="pst")
                nc.tensor.transpose(gtp, ge, id32[0:G, 0:G])
                gt16 = ph2.tile([A, G], BF16, tag="gt16")
                nc.vector.tensor_copy(out=gt16, in_=gtp)

            # ---- h' = ge*relu(h) -> transpose -> fc2, pipelined per chunk.
            # PE executes in emission order, so interleave the transposes with
            # the fc2 matmuls; relu and psum->sbuf copies run on ACT so DVE
            # stays free for next group's pools.
            h16 = ph1.tile([G, ACH], BF16, tag="h16")
            hT = pp.tile([128, (ACH // 128) * G], BF16, tag="hT")
            mps = psm.tile([G, C], F32, tag="mps")
            nt = ACH // 128
            for t0 in range(0, nt, TPB_COPY):
                for t in range(t0, t0 + TPB_COPY):
                    if t % 2 == 0:
                        a = t // 2
                        nc.scalar.activation(
                            out=h16[:, a * CH : (a + 1) * CH],
                            in_=hps[:, a * CH : (a + 1) * CH],
                            func=AF.Relu,
                            scale=ge[:, a : a + 1],
                        )
                tp = pss.tile([128, TPB_COPY * G], BF16, tag="pst")
                for j in range(TPB_COPY):
                    nc.tensor.transpose(
                        tp[:, j * G : (j + 1) * G],
                        h16[:, (t0 + j) * 128 : (t0 + j + 1) * 128],
                        id16[0:G, 0:G],
                    )
                nc.scalar.activation(
                    out=hT[:, t0 * G : (t0 + TPB_COPY) * G], in_=tp, func=AF.Copy
                )
                for t in range(t0, t0 + TPB_COPY):
                    for n in range(C // 512):
                        nc.tensor.matmul(
                            out=mps[:, n * 512 : (n + 1) * 512],
                            lhsT=hT[:, t * G : (t + 1) * G],
                            rhs=w2_sb[:, t, n * 512 : (n + 1) * 512],
                            start=(t == 0),
                            stop=(t == nt - 1) and not with_bias,
                        )
            if with_bias:
                for n in range(C // 512):
                    nc.tensor.matmul(
                        out=mps[:, n * 512 : (n + 1) * 512],
                        lhsT=gt16,
                        rhs=b2_sb[:, n * 512 : (n + 1) * 512],
                        start=False,
                        stop=True,
                    )

            # ---- scale = 1 + sigmoid(mixed) = 1.5 + 0.5*tanh(mixed/2).
            # w2 is host-scaled by 0.5, so mps = mixed_unnorm/2 and the
            # per-sample softmax denominator folds into the tanh scale.
            scl = pp.tile([128, K8 * G], F32, tag="scl")
            sp = pss.tile([128, K8 * G], BF16, tag="pst")
            mx = ph1.tile([G, C], BF16, tag="mx")
            nc.scalar.activation(out=mx, in_=mps, func=AF.Tanh, scale=gi[:, 0:1])
            mxv = mx.rearrange("g (p k) -> g k p", k=K8)  # [G, 8, 128]
            for k in range(K8):
                nc.tensor.transpose(
                    sp[:, k * G : (k + 1) * G], mxv[:, k, :], id16[0:G, 0:G]
                )
            nc.scalar.activation(
                out=scl, in_=sp, func=AF.Copy, scale=0.5, bias=1.5
            )

            # ---- apply scale in place, store.
            # ACT applies samples 0..3 (tiles 0,1); GpSimd applies samples
            # 4..7 (tiles 2,3), concurrently.  Each engine dispatches the
            # stores of the OTHER engine's samples, one sample behind, so a
            # store's apply has always completed by the time the dispatching
            # sequencer reaches it (an unmet DMA wait holds the sequencer).
            # Loads live alone on the SP ring; ACT stores use the second
            # HWDGE ring; GpSimd stores go out via SWDGE.
            def _apply(eng, bcol):
                t, bb = divmod(bcol, TPB)
                for k in range(K8):
                    sl = ftiles[t][:, bb, k, :]
                    s1 = scl[:, k * G + bcol : k * G + bcol + 1]
                    if eng == "act":
                        nc.scalar.activation(out=sl, in_=sl, func=AF.Copy, scale=s1)
                    elif eng == "dve":
                        nc.vector.tensor_scalar_mul(out=sl, in0=sl, scalar1=s1)
                    else:
                        nc.gpsimd.tensor_scalar_mul(out=sl, in0=sl, scalar1=s1)

            def _store(eng, bcol):
                t, bb = divmod(bcol, TPB)
                dma = {
                    "act": nc.scalar.dma_start,
                    "sync": nc.sync.dma_start,
                    "pool": nc.gpsimd.dma_start,
                }[eng]
                dma(out=ov[:, g * G + bcol], in_=ftiles[t][:, bb])

            HALF = G // 2
            if SCHEME == "v6":
                # v1 but with a fast wave turn-on: sample 0's apply is split
                # across ACT and GpSimd (parallel halves, ~1.5us), and its
                # store dispatches from GpSimd's SWDGE immediately, ~3us
                # after scl instead of ~5.5.  Remaining samples follow v1.
                t0, bb0 = divmod(0, TPB)
                for k in range(K8):
                    sl = ftiles[t0][:, bb0, k, :]
                    s1 = scl[:, k * G : k * G + 1]
                    if k < K8 // 2:
                        nc.scalar.activation(out=sl, in_=sl, func=AF.Copy, scale=s1)
                    else:
                        nc.gpsimd.tensor_scalar_mul(out=sl, in0=sl, scalar1=s1)
                _store("pool", 0)
                for b in range(1, G):
                    _apply("act" if b < HALF else "pool", b)
                    _store("act", b)
            elif SCHEME == "v7":
                # v6's split first-apply + ACT/GpSimd apply halves, but all
                # stores dispatch from the SP ring like the original kernel
                # (ACT-issued stores measured pathologically slow on HW).
                t0, bb0 = divmod(0, TPB)
                for k in range(K8):
                    sl = ftiles[t0][:, bb0, k, :]
                    s1 = scl[:, k * G : k * G + 1]
                    if k < K8 // 2:
                        nc.scalar.activation(out=sl, in_=sl, func=AF.Copy, scale=s1)
                    else:
                        nc.gpsimd.tensor_scalar_mul(out=sl, in0=sl, scalar1=s1)
                _store("sync", 0)
                for b in range(1, G):
                    _apply("act" if b < HALF else "pool", b)
                    _store("sync", b)
            elif SCHEME == "v2":
                # ACT applies 0..3, GpSimd applies 4..7; each stores the
                # other's samples one sample behind.
                for i in range(HALF):
                    _apply("act", i)
                    _apply("pool", HALF + i)
                    if i >= 1:
                        _store("act", HALF + i - 1)
                        _store("pool", i - 1)
                _store("act", 2 * HALF - 1)
                _store("pool", HALF - 1)
            elif SCHEME == "v1":
                # ACT applies+stores 0..3, GpSimd applies 4..7, ACT stores them.
                for b in range(G):
                    _apply("act" if b < HALF else "pool", b)
                    _store("act", b)
            elif SCHEME == "base":
                # DVE + ACT applies (GpSimd compute measured pathologically
                # slow on HW), stores on the sync ring.  NDVE picks how many
                # samples apply on DVE; ACT (which also runs the MLP glue)
                # takes the rest.  With FSPLIT, sample 0's eight slices are
                # split across DVE and ACT so the wave's first store
                # dispatches ~1.4us after scl instead of ~2.8us.
                start_b = 0
                if FSPLIT:
                    for k in range(K8):
                        sl = ftiles[0][:, 0, k, :]
                        s1 = scl[:, k * G : k * G + 1]
                        if k < K8 // 2:
                            nc.vector.tensor_scalar_mul(out=sl, in0=sl, scalar1=s1)
                        else:
                            nc.scalar.activation(out=sl, in_=sl, func=AF.Copy, scale=s1)
                    if PB_STORES:
                        _store("sync", 0)
                    start_b = 1
                for b in range(start_b, G):
                    _apply("dve" if b < NDVE else "act", b)
                    if PB_STORES:
                        _store("sync", b)
                if not PB_STORES:
                    sdma = {
                        "sync": nc.sync.dma_start,
                        "act": nc.scalar.dma_start,
                        "pool": nc.gpsimd.dma_start,
                    }[SRING]
                    for t in range(TPG):
                        b0 = g * G + t * TPB
                        sdma(out=ov[:, b0 : b0 + TPB], in_=ftiles[t])
            elif SCHEME == "v4":
                # ACT applies s0..3, GpSimd applies s4..7.  GpSimd's SWDGE
                # dispatches the stores of ACT's samples interleaved with its
                # own applies (first store fires ~3us after scl); ACT
                # dispatches the stores of GpSimd's samples at its tail.
                _store("pool", 0)
                for i in range(HALF):
                    _apply("act", i)
                    _apply("pool", HALF + i)
                    if i >= 1:
                        _store("pool", i)
                for i in range(HALF):
                    _store("act", HALF + i)
            elif SCHEME == "v3":
                # like v2 but stores dispatch from the engine that applied,
                # one sample behind its own stream (exec-queue drain hidden).
                for i in range(HALF):
                    _apply("act", i)
                    _apply("pool", HALF + i)
                    if i >= 1:
                        _store("act", i - 1)
                        _store("pool", HALF + i - 1)
                _store("act", HALF - 1)
                _store("pool", 2 * HALF - 1)
            else:
                raise ValueError(SCHEME)


def build_nc(with_bias=False, repeat=1):
    nc = bacc.Bacc(
        "TRN2",
        target_bir_lowering=False,
        debug=not axon_active(),
        num_devices=NCORES,
    )
    fshape = [128, BS, K8, HW] if REPACK else [BS, C, HW]
    feat = nc.dram_tensor("features", fshape, FDT, kind="ExternalInput").ap()
    w1 = nc.dram_tensor("w1", [128, K8, ACH], WDT, kind="ExternalInput").ap()
    wg = nc.dram_tensor("wg", [128, K8, A], BF16, kind="ExternalInput").ap()
    w2 = nc.dram_tensor("w2", [128, ACH // 128, C], WDT, kind="ExternalInput").ap()
    b1h = b1g = b2 = None
    if with_bias:
        b1h = nc.dram_tensor("b1h", [1, ACH], BF16, kind="ExternalInput").ap()
        b1g = nc.dram_tensor("b1g", [1, A], BF16, kind="ExternalInput").ap()
        b2 = nc.dram_tensor("b2", [A, C], BF16, kind="ExternalInput").ap()
    out = nc.dram_tensor("out", fshape, FDT, kind="ExternalOutput").ap()

    with tile.TileContext(nc) as tc:
        _emit(tc, nc, feat, w1, wg, w2, b1h, b1g, b2, out, with_bias, repeat)
    nc.compile()
    return nc


def _get_nc(with_bias=False):
    key = "bias" if with_bias else "nobias"
    if key not in _NC_CACHE:
        _NC_CACHE[key] = build_nc(with_bias)
    return _NC_CACHE[key]


def prep_host_inputs(features, fc_w, fc_b, fc1_w, fc1_b, fc2_w, fc2_b):
    """Returns (per-core features list, shared weight dict, with_bias)."""
    bf16 = ml_dtypes.bfloat16
    wdt = mybir.dt.np(WDT)

    fc1_w = np.asarray(fc1_w, dtype=np.float32)
    fc_w = np.asarray(fc_w, dtype=np.float32)
    fc2_w = np.asarray(fc2_w, dtype=np.float32)
    fc_b = np.asarray(fc_b, dtype=np.float32)
    fc1_b = np.asarray(fc1_b, dtype=np.float32)
    fc2_b = np.asarray(fc2_b, dtype=np.float32)
    with_bias = bool(np.any(fc_b) or np.any(fc1_b) or np.any(fc2_b))

    # The 1/HW spatial-mean factor is applied on-chip at the P16 copy, so
    # the weights stay at their natural scale (fp8-safe).
    # w1[p, k, a*CH+ch] = fc1_w[a, 8p+k, ch]
    w1 = (
        np.transpose(fc1_w, (1, 0, 2)).reshape(C, ACH).reshape(128, K8, ACH)
    ).astype(wdt)
    # wg[p, k, a] = fc_w[a, 8p+k]
    wg = (fc_w.T.reshape(128, K8, A)).astype(bf16)
    # w2[p, t, c] = fc2_w.reshape(ACH, C)[t*128+p, c] * 0.5 -- the 0.5 of
    # tanh(mixed/2) is folded into w2 so the softmax denominator 1/gs is the
    # only runtime factor in the tanh scale.
    w2 = np.ascontiguousarray(
        fc2_w.reshape(ACH, C).reshape(ACH // 128, 128, C).transpose(1, 0, 2) * 0.5
    ).astype(wdt)
    weights = {
        "w1": np.ascontiguousarray(w1),
        "wg": np.ascontiguousarray(wg),
        "w2": w2,
    }
    if with_bias:
        weights["b1h"] = fc1_b.reshape(1, ACH).astype(bf16)
        weights["b1g"] = fc_b.reshape(1, A).astype(bf16)
        weights["b2"] = (fc2_b * 0.5).astype(bf16)
    f = np.ascontiguousarray(np.asarray(features, dtype=np.float32)).reshape(B, C, HW)
    if FDT is not F32:
        f = f.astype(mybir.dt.np(FDT))
    if REPACK:
        # [BS, C, HW] -> [128, BS, K8, HW]: partition-major so each
        # (partition, sample-range) is one contiguous DRAM run.
        shards = [
            np.ascontiguousarray(
                f[i * BS : (i + 1) * BS]
                .reshape(BS, 128, K8, HW)
                .transpose(1, 0, 2, 3)
            )
            for i in range(NCORES)
        ]
    else:
        shards = [f[i * BS : (i + 1) * BS] for i in range(NCORES)]
    return shards, weights, with_bias


def run(inputs, trace=False, trace_kwargs=None):
    shards, weights, with_bias = prep_host_inputs(**inputs)
    nc = _get_nc(with_bias)
    in_maps = [dict(weights, features=shards[i]) for i in range(NCORES)]
    res = run_bass_kernel_spmd(
        nc,
        in_maps,
        core_ids=list(range(NCORES)),
        trace=trace,
        **(trace_kwargs or {}),
    )
    return assemble_out([res.results[i]["out"] for i in range(NCORES)]), res


def assemble_out(core_outs):
    """Per-core device 'out' arrays -> full [B, C, H, W] array."""
    if REPACK:
        out = np.concatenate(
            [
                np.asarray(o).transpose(1, 0, 2, 3).reshape(BS, C, HW)
                for o in core_outs
            ],
            axis=0,
        )
    else:
        out = np.concatenate([np.asarray(o) for o in core_outs], axis=0)
    return out.reshape(B, C, H, W)


def kernel(**inputs):
    out, _ = run(inputs, trace=False)
    return np.asarray(out, dtype=np.float32)

